# revision 6
# baseline (speedup 1.0000x reference)
"""Trainium2 Bass kernel for nn_DetailDecoder (8-layer transformer, B=8, T=1024).

Sharding: pure data-parallel over batch B=8 across the 8 NeuronCores (one
sequence per core, weights replicated, zero collectives).

Per-core layout strategy (T=1024 tokens):
  - residual x token-major [128p tok, 8 tiles, 512 feat]
  - LN gains/biases, 1/sqrt(d_head) and per-head ALiBi scales are folded into
    the weights on the host, so device LN is pure (x-m)*rstd
  - matmul inputs are produced feature-major via PE transposes
  - attention is computed key-major: scoresT[tk,tq] = K_fm.T @ Q_fm in PSUM,
    plus a (-I) @ fl matmul accumulating the periodic distance bias; Exp()
    evicts straight to probability tiles PT[tk,tq] (bf16), so attn@V needs no
    P transpose; an all-ones column appended to V yields the softmax
    denominator for free.
"""
import numpy as np
import ml_dtypes

import concourse.bass as bass
import concourse.tile as tile
import concourse.mybir as mybir
from concourse import bacc

F32 = mybir.dt.float32
F32R = mybir.dt.float32r
BF16 = mybir.dt.bfloat16
AF = mybir.ActivationFunctionType
ALU = mybir.AluOpType

B, T = 8, 1024
D_MOTION = 135
D_MODEL, N_HEADS, D_HEAD, D_FF, N_LAYERS, PERIOD = 512, 8, 64, 2048, 8, 30
D_IN = D_MOTION + 1 + 5          # 141
D_OUT = D_MOTION + 4             # 139
D_OUT_P = D_OUT + 1              # 140 (fp32r matmul needs even free count)
NT = T // 128                    # 8 token tiles
NK = D_MODEL // 128              # 4 feature chunks
NF = D_FF // 128                 # 16 ff tiles
EPS = 1e-5
N_CORES = 8


# --------------------------------------------------------------------------
# host-side input preparation
# --------------------------------------------------------------------------

def _host_prep(motion, traj, mask, p):
    f32 = np.float32
    g_a = np.asarray(p['attn_ln_g'], f32);  b_a = np.asarray(p['attn_ln_b'], f32)
    g_f = np.asarray(p['ffn_ln_g'], f32);   b_f = np.asarray(p['ffn_ln_b'], f32)
    g_F = np.asarray(p['final_ln_g'], f32); b_F = np.asarray(p['final_ln_b'], f32)

    wq = np.asarray(p['Wq'], f32); bq = np.asarray(p['bq'], f32)
    wk = np.asarray(p['Wk'], f32); bk = np.asarray(p['bk'], f32)
    wv = np.asarray(p['Wv'], f32); bv = np.asarray(p['bv'], f32)
    wo = np.asarray(p['Wo'], f32); bo = np.asarray(p['bo'], f32)
    w1 = np.asarray(p['ff_W1'], f32); b1 = np.asarray(p['ff_b1'], f32)
    w2 = np.asarray(p['ff_W2'], f32); b2 = np.asarray(p['ff_b2'], f32)

    inv_sqrt = f32(1.0 / np.sqrt(D_HEAD))
    # per-head ALiBi scale 2^-(h+1) folded into Wq columns (exact powers of 2);
    # Exp() applies it back via its scale operand.
    headscale = np.repeat(2.0 ** np.arange(1, N_HEADS + 1), D_HEAD).astype(f32)

    wq_f = np.empty_like(wq); bq_f = np.empty_like(bq)
    wk_f = np.empty_like(wk); bk_f = np.empty_like(bk)
    wv_f = np.empty_like(wv); bv_f = np.empty_like(bv)
    w1_f = np.empty_like(w1); b1_f = np.empty_like(b1)
    for l in range(N_LAYERS):
        wq_f[l] = (g_a[l][:, None] * wq[l]) * inv_sqrt * headscale[None, :]
        bq_f[l] = (b_a[l] @ wq[l] + bq[l]) * inv_sqrt * headscale
        wk_f[l] = g_a[l][:, None] * wk[l]
        bk_f[l] = b_a[l] @ wk[l] + bk[l]
        wv_f[l] = g_a[l][:, None] * wv[l]
        bv_f[l] = b_a[l] @ wv[l] + bv[l]
        w1_f[l] = g_f[l][:, None] * w1[l]
        b1_f[l] = b_f[l] @ w1[l] + b1[l]

    dw1 = np.asarray(p['dec_W1'], f32); db1 = np.asarray(p['dec_b1'], f32)
    dw1_f = g_F[:, None] * dw1
    db1_f = b_F @ dw1 + db1

    # periodic positional encoding [T, D_MODEL]
    pos = (np.arange(T) % PERIOD).astype(f32)
    half = np.arange(0, D_MODEL, 2, dtype=f32)
    div = np.exp(-np.log(10000.0) * half / D_MODEL)
    ang = pos[:, None] * div[None, :]
    pe = np.zeros((T, D_MODEL), f32)
    pe[:, 0::2] = np.sin(ang)
    pe[:, 1::2] = np.cos(ang)

    # periodic frame-distance staircase [T, T] (head-independent part)
    fr = np.arange(T, dtype=f32)
    fl = np.floor(np.abs(fr[None, :] - fr[:, None]) / PERIOD)

    shared = {
        'wq': wq_f, 'wk': wk_f, 'wv': wv_f,
        'wobf': wo.astype(ml_dtypes.bfloat16),
        'w1bf': w1_f.astype(ml_dtypes.bfloat16),
        'w2bf': w2.astype(ml_dtypes.bfloat16),
        'bq': bq_f, 'bk': bk_f, 'b1': b1_f,
        'bvr': bv_f, 'bor': bo, 'b2r': np.broadcast_to(b2, (N_LAYERS, D_MODEL)).copy(),
        'encw1': np.asarray(p['enc_W1'], f32),
        'encw2': np.asarray(p['enc_W2'], f32),
        'encb1': np.asarray(p['enc_b1'], f32),
        'encb2r': np.asarray(p['enc_b2'], f32)[None],
        'decw1': dw1_f, 'decw2': np.pad(np.asarray(p['dec_W2'], f32), ((0, 0), (0, 1))),
        'decb1': db1_f, 'decb2r': np.pad(np.asarray(p['dec_b2'], f32), (0, 1))[None],
        'pe': pe, 'fl': fl.astype(ml_dtypes.bfloat16),
        'nident': (-np.eye(128, dtype=f32)).astype(ml_dtypes.bfloat16),
        'ident': np.eye(128, dtype=f32),
        'onesr': np.ones((1, 128), f32),
        'alpham1': np.array([[p['enc_a1'] - 1.0, p['enc_a2'] - 1.0,
                              p['dec_a1'] - 1.0]], f32),
    }

    motion = np.asarray(motion, f32)
    traj = np.asarray(traj, f32)
    mask = np.asarray(mask, f32)
    per_core = []
    for b in range(B):
        xt = np.concatenate([motion[b], traj[b], mask[b]], axis=-1).T
        per_core.append({
            'xT': np.ascontiguousarray(xt),                 # [141, 1024]
            'motion': np.ascontiguousarray(motion[b]),      # [1024, 135]
            'mask_p': np.ascontiguousarray(mask[b, :, 0]),  # [1024]
        })
    return shared, per_core


# --------------------------------------------------------------------------
# device program
# --------------------------------------------------------------------------

def build_nc():
    nc = bacc.Bacc(None, target_bir_lowering=False, debug=False)
    dp = nc.declare_dram_parameter
    d = {
        'xT': dp('xT', [D_IN, T], F32R, isOutput=False),
        'onesr': dp('onesr', [1, 128], F32R, isOutput=False),
        'motion': dp('motion', [T, D_MOTION], F32, isOutput=False),
        'mask_p': dp('mask_p', [T], F32, isOutput=False),
        'pe': dp('pe', [T, D_MODEL], F32, isOutput=False),
        'fl': dp('fl', [T, T], BF16, isOutput=False),
        'nident': dp('nident', [128, 128], BF16, isOutput=False),
        'ident': dp('ident', [128, 128], F32, isOutput=False),
        'alpham1': dp('alpham1', [1, 3], F32, isOutput=False),
        'wq': dp('wq', [N_LAYERS, D_MODEL, D_MODEL], F32R, isOutput=False),
        'wk': dp('wk', [N_LAYERS, D_MODEL, D_MODEL], F32R, isOutput=False),
        'wv': dp('wv', [N_LAYERS, D_MODEL, D_MODEL], F32R, isOutput=False),
        'wobf': dp('wobf', [N_LAYERS, D_MODEL, D_MODEL], BF16, isOutput=False),
        'w1bf': dp('w1bf', [N_LAYERS, D_MODEL, D_FF], BF16, isOutput=False),
        'w2bf': dp('w2bf', [N_LAYERS, D_FF, D_MODEL], BF16, isOutput=False),
        'bq': dp('bq', [N_LAYERS, D_MODEL], F32, isOutput=False),
        'bk': dp('bk', [N_LAYERS, D_MODEL], F32, isOutput=False),
        'b1': dp('b1', [N_LAYERS, D_FF], F32, isOutput=False),
        'bvr': dp('bvr', [N_LAYERS, D_MODEL], F32R, isOutput=False),
        'bor': dp('bor', [N_LAYERS, D_MODEL], F32R, isOutput=False),
        'b2r': dp('b2r', [N_LAYERS, D_MODEL], F32R, isOutput=False),
        'encw1': dp('encw1', [D_IN, D_MODEL], F32R, isOutput=False),
        'encw2': dp('encw2', [D_MODEL, D_MODEL], F32R, isOutput=False),
        'encb1': dp('encb1', [D_MODEL], F32, isOutput=False),
        'encb2r': dp('encb2r', [1, D_MODEL], F32R, isOutput=False),
        'decw1': dp('decw1', [D_MODEL, D_MODEL], F32R, isOutput=False),
        'decw2': dp('decw2', [D_MODEL, D_OUT_P], F32R, isOutput=False),
        'decb1': dp('decb1', [D_MODEL], F32, isOutput=False),
        'decb2r': dp('decb2r', [1, D_OUT_P], F32R, isOutput=False),
        'y': dp('y', [T, D_OUT], F32, isOutput=True),
    }
    with tile.TileContext(nc) as tc:
        _emit(nc, tc, d)
    return nc


def _emit(nc, tc, d):
    import contextlib
    ctx = contextlib.ExitStack()
    with ctx:
        consts = ctx.enter_context(tc.tile_pool(name="consts", bufs=1))
        state = ctx.enter_context(tc.tile_pool(name="state", bufs=1))
        fmp = ctx.enter_context(tc.tile_pool(name="fmp", bufs=1))
        qp = ctx.enter_context(tc.tile_pool(name="qp", bufs=1))
        kp = ctx.enter_context(tc.tile_pool(name="kp", bufs=1))
        vp = ctx.enter_context(tc.tile_pool(name="vp", bufs=1))
        ptp = ctx.enter_context(tc.tile_pool(name="ptp", bufs=1))
        op = ctx.enter_context(tc.tile_pool(name="op", bufs=1))
        hp = ctx.enter_context(tc.tile_pool(name="hp", bufs=1))
        wqkvo = ctx.enter_context(tc.tile_pool(name="wqkvo", bufs=2))
        ewbp = ctx.enter_context(tc.tile_pool(name="ewbp", bufs=1))
        w1p = ctx.enter_context(tc.tile_pool(name="w1p", bufs=1))
        w2p = ctx.enter_context(tc.tile_pool(name="w2p", bufs=1))
        rows = ctx.enter_context(tc.tile_pool(name="rows", bufs=1))
        sst = ctx.enter_context(tc.tile_pool(name="sst", bufs=6))
        scr = ctx.enter_context(tc.tile_pool(name="scr", bufs=2))
        rlp = ctx.enter_context(tc.tile_pool(name="rlp", bufs=1))
        io = ctx.enter_context(tc.tile_pool(name="io", bufs=2))
        ps_a = ctx.enter_context(tc.tile_pool(name="ps_a", bufs=3, space="PSUM"))
        ps_s = ctx.enter_context(tc.tile_pool(name="ps_s", bufs=2, space="PSUM"))

        dma = nc.sync.dma_start

        # ---------------- constants ----------------
        fl_s = consts.tile([128, NT, T], BF16)
        dma(out=fl_s[:], in_=d['fl'][:].rearrange("(i p) q -> p i q", p=128))
        nident_s = consts.tile([128, 128], BF16)
        dma(out=nident_s[:], in_=d['nident'][:])
        ident_s = consts.tile([128, 128], F32)
        dma(out=ident_s[:], in_=d['ident'][:])
        bq_s = consts.tile([128, N_LAYERS, NK], F32)
        dma(out=bq_s[:], in_=d['bq'][:].rearrange("l (t p) -> p l t", p=128))
        bk_s = consts.tile([128, N_LAYERS, NK], F32)
        dma(out=bk_s[:], in_=d['bk'][:].rearrange("l (t p) -> p l t", p=128))
        b1_s = consts.tile([128, N_LAYERS, NF], F32)
        dma(out=b1_s[:], in_=d['b1'][:].rearrange("l (t p) -> p l t", p=128))
        encb1_s = consts.tile([128, NK], F32)
        dma(out=encb1_s[:], in_=d['encb1'][:].rearrange("(t p) -> p t", p=128))
        decb1_s = consts.tile([128, NK], F32)
        dma(out=decb1_s[:], in_=d['decb1'][:].rearrange("(t p) -> p t", p=128))
        alpham1_s = consts.tile([128, 3], F32)
        dma(out=alpham1_s[:], in_=d['alpham1'][:].to_broadcast([128, 3]))
        mask_s = consts.tile([128, NT], F32)
        dma(out=mask_s[:], in_=d['mask_p'][:].rearrange("(i p) -> p i", p=128))
        ones_s = consts.tile([1, 128], F32R)
        dma(out=ones_s[:], in_=d['onesr'][:])
        eps_s = consts.tile([128, 1], F32)
        nc.vector.memset(eps_s[:], EPS)

        x_s = state.tile([128, NT, D_MODEL], F32)    # residual stream (tm)

        # ---------------- helpers ----------------
        def layernorm_to_fm(dst_fm):
            """dst_fm[128, NK, T] <- transpose(normalize(x)) for all 8 tiles."""
            for i in range(NT):
                xt = x_s[:, i, :]
                st = sst.tile([128, 6], F32, tag="st")
                mv = sst.tile([128, 2], F32, tag="mv")
                nc.vector.bn_stats(out=st[:], in_=xt)
                nc.vector.bn_aggr(out=mv[:], in_=st[:])
                sd = sst.tile([128, 1], F32, tag="sd")
                nc.scalar.activation(out=sd[:], in_=mv[:, 1:2], func=AF.Sqrt,
                                     bias=eps_s[:, 0:1], scale=1.0)
                rstd = sst.tile([128, 1], F32, tag="rstd")
                nc.vector.reciprocal(out=rstd[:], in_=sd[:])
                nmr = sst.tile([128, 1], F32, tag="nmr")
                nc.vector.scalar_tensor_tensor(
                    out=nmr[:], in0=mv[:, 0:1], scalar=-1.0, in1=rstd[:],
                    op0=ALU.mult, op1=ALU.mult)
                xi = scr.tile([128, D_MODEL], F32, tag="xi")
                nc.scalar.activation(out=xi[:], in_=xt, func=AF.Identity,
                                     bias=nmr[:, 0:1], scale=rstd[:, 0:1])
                tr = ps_a.tile([128, NK, 128], F32, tag="tr")
                for k in range(NK):
                    nc.tensor.transpose(tr[:, k, :], xi[:, k * 128:(k + 1) * 128],
                                        ident_s[:])
                nc.scalar.copy(out=dst_fm[:, :, i * 128:(i + 1) * 128], in_=tr[:])

        def load_w(pool, src_ap, kn, n, dtype=F32, tag="w"):
            wt = pool.tile([128, kn, n], dtype, tag=tag)
            dma(out=wt[:], in_=src_ap.rearrange("(k p) n -> p k n", p=128))
            return wt

        def proj_fm(dst_fm, src_fm, wt, bias_fn):
            """dst_fm[128,NK,T] = wt.T @ src_fm (+ per-partition bias)."""
            for m in range(NK):
                for j in range(2):
                    ps = ps_a.tile([128, 512], F32, tag="pj")
                    for k in range(NK):
                        nc.tensor.matmul(
                            ps[:], wt[:, k, m * 128:(m + 1) * 128],
                            src_fm[:, k, j * 512:(j + 1) * 512],
                            start=(k == 0), stop=(k == NK - 1))
                    nc.scalar.activation(
                        out=dst_fm[:, m, j * 512:(j + 1) * 512], in_=ps[:],
                        func=AF.Identity, bias=bias_fn(m), scale=1.0)

        def bias_row(src_ap):
            """Stage a [1, n] bias row in SBUF for the ones-row matmul."""
            n = src_ap.shape[-1]
            rt = rows.tile([1, 512], F32R, tag="br")
            dma(out=rt[:, 0:n], in_=src_ap)
            return rt[:, 0:n]

        def prelu_from_psum(dst, ps_in, alpha_ap, bias_ap=None):
            """dst = prelu(ps_in + bias) = t + (a-1)*min(t,0)."""
            tt = scr.tile([128, 512], F32, tag="xi")
            if bias_ap is not None:
                nc.scalar.activation(out=tt[:], in_=ps_in, func=AF.Identity,
                                     bias=bias_ap, scale=1.0)
            else:
                nc.scalar.copy(out=tt[:], in_=ps_in)
            mt = scr.tile([128, 512], F32, tag="mt")
            nc.vector.tensor_scalar_min(mt[:], tt[:], 0.0)
            nc.vector.scalar_tensor_tensor(
                out=dst, in0=mt[:], scalar=alpha_ap, in1=tt[:],
                op0=ALU.mult, op1=ALU.add)
            return tt

        # ---------------- encoder ----------------
        xa = hp.tile([128, T], F32R, tag="h")
        dma(out=xa[:], in_=d['xT'][0:128, :])
        xb = vp.tile([13, T], F32R, tag="v")
        dma(out=xb[:], in_=d['xT'][128:141, :])
        ew1a = load_w(wqkvo, d['encw1'][0:128, :], 1, D_MODEL, dtype=F32R, tag="w")
        ew1b = ewbp.tile([13, D_MODEL], F32R)
        dma(out=ew1b[:], in_=d['encw1'][128:141, :])

        x1_fm = fmp.tile([128, NK, T], F32R, tag="fm")
        for m in range(NK):
            for j in range(2):
                ps = ps_a.tile([128, 512], F32, tag="pj")
                nc.tensor.matmul(ps[:],
                                 ew1a[:, 0, m * 128:(m + 1) * 128],
                                 xa[:, j * 512:(j + 1) * 512],
                                 start=True, stop=False)
                nc.tensor.matmul(ps[:],
                                 ew1b[:, m * 128:(m + 1) * 128],
                                 xb[:, j * 512:(j + 1) * 512],
                                 start=False, stop=True)
                prelu_from_psum(x1_fm[:, m, j * 512:(j + 1) * 512], ps[:],
                                alpham1_s[:, 0:1], encb1_s[:, m:m + 1])

        ew2 = load_w(wqkvo, d['encw2'][:], NK, D_MODEL, dtype=F32R, tag="w")
        encb2 = bias_row(d['encb2r'][0:1, :])
        for i in range(NT):
            ps = ps_a.tile([128, 512], F32, tag="pj")
            for k in range(NK):
                nc.tensor.matmul(ps[:],
                                 x1_fm[:, k, i * 128:(i + 1) * 128],
                                 ew2[:, k, :],
                                 start=(k == 0), stop=False)
            nc.tensor.matmul(ps[:], ones_s[:],
                             encb2, start=False, stop=True)
            x2 = scr.tile([128, 512], F32, tag="mt")
            prelu_from_psum(x2[:], ps[:], alpham1_s[:, 1:2])
            pe_t = io.tile([128, D_MODEL], F32, tag="pe")
            dma(out=pe_t[:], in_=d['pe'][i * 128:(i + 1) * 128, :])
            nc.vector.tensor_add(x_s[:, i, :], x2[:], pe_t[:])

        # ---------------- transformer layers ----------------
        for l in range(N_LAYERS):
            # --- attention ---
            xi_fm = fmp.tile([128, NK, T], F32R, tag="fm")
            layernorm_to_fm(xi_fm)

            wq_t = load_w(wqkvo, d['wq'][l], NK, D_MODEL, dtype=F32R, tag="w")
            q_fm = qp.tile([128, NK, T], F32R)
            proj_fm(q_fm, xi_fm, wq_t, lambda m: bq_s[:, l, m:m + 1])

            wk_t = load_w(wqkvo, d['wk'][l], NK, D_MODEL, dtype=F32R, tag="w")
            k_fm = kp.tile([128, NK, T], F32R)
            proj_fm(k_fm, xi_fm, wk_t, lambda m: bk_s[:, l, m:m + 1])

            # V (token-major, bf16, ones column for the softmax denominator)
            wv_t = load_w(wqkvo, d['wv'][l], NK, D_MODEL, dtype=F32R, tag="w")
            v_s = vp.tile([128, NT, N_HEADS, D_HEAD + 1], BF16, tag="v")
            nc.vector.memset(v_s[:], 1.0)
            bvrow = bias_row(d['bvr'][l:l + 1, :])
            for i in range(NT):
                ps = ps_a.tile([128, 512], F32, tag="pj")
                for k in range(NK):
                    nc.tensor.matmul(
                        ps[:], xi_fm[:, k, i * 128:(i + 1) * 128],
                        wv_t[:, k, :],
                        start=(k == 0), stop=False)
                nc.tensor.matmul(ps[:], ones_s[:],
                                 bvrow, start=False, stop=True)
                nc.scalar.copy(out=v_s[:, i, :, 0:D_HEAD],
                               in_=ps[:].rearrange("p (h e) -> p h e", h=N_HEADS))

            # --- scores + softmax + attn@V, head by head ---
            o_fm = op.tile([128, NK, T], BF16)
            for h in range(N_HEADS):
                hb = (h % 2) * 64
                hm = h // 2
                s_h = float(2.0 ** (-(h + 1)))
                for j in range(2):
                    pt = ptp.tile([128, NT, 512], BF16, tag="pt")
                    for tk in range(NT):
                        sps = ps_s.tile([128, 512], F32, tag="s")
                        nc.tensor.matmul(
                            sps[:],
                            k_fm[hb:hb + 64, hm, tk * 128:(tk + 1) * 128],
                            q_fm[hb:hb + 64, hm, j * 512:(j + 1) * 512],
                            start=True, stop=False)
                        nc.tensor.matmul(
                            sps[:], nident_s[:],
                            fl_s[:, tk, j * 512:(j + 1) * 512],
                            start=False, stop=True)
                        nc.scalar.activation(out=pt[:, tk, :], in_=sps[:],
                                             func=AF.Exp, bias=0.0, scale=s_h)
                    ops = ps_a.tile([128, 512], F32, tag="pj")
                    for tk in range(NT):
                        nc.tensor.matmul(ops[0:65, :], v_s[:, tk, h, :],
                                         pt[:, tk, :],
                                         start=(tk == 0), stop=(tk == NT - 1))
                    rl = rlp.tile([1, 512], F32, tag="rl")
                    nc.vector.reciprocal(out=rl[:], in_=ops[64:65, :])
                    rlb = rlp.tile([64, 512], F32, tag="rlb")
                    nc.gpsimd.partition_broadcast(rlb[:], rl[:])
                    nc.vector.tensor_mul(
                        o_fm[hb:hb + 64, hm, j * 512:(j + 1) * 512],
                        ops[0:64, :], rlb[:])

            # --- output projection + residual ---
            wo_t = load_w(wqkvo, d['wobf'][l], NK, D_MODEL, dtype=BF16, tag="w")
            borow = bias_row(d['bor'][l:l + 1, :])
            for i in range(NT):
                ps = ps_a.tile([128, 512], F32, tag="pj")
                for k in range(NK):
                    nc.tensor.matmul(
                        ps[:], o_fm[:, k, i * 128:(i + 1) * 128],
                        wo_t[:, k, :],
                        start=(k == 0), stop=False)
                nc.tensor.matmul(ps[:], ones_s[:],
                                 borow, start=False, stop=True)
                nc.vector.tensor_add(x_s[:, i, :], x_s[:, i, :], ps[:])

            # --- FFN (bf16 weights/activations, f32 accumulation) ---
            xf_fm = fmp.tile([128, NK, T], BF16, tag="fm")
            layernorm_to_fm(xf_fm)

            w1_t = load_w(w1p, d['w1bf'][l], NK, D_FF, dtype=BF16, tag="w1")
            w2_t = load_w(w2p, d['w2bf'][l], NF, D_MODEL, dtype=BF16, tag="w2")
            b2row = bias_row(d['b2r'][l:l + 1, :])
            for jq in range(4):            # token quarters
                h_s = hp.tile([128, NF, 256], BF16, tag="h")
                for f in range(NF):
                    ps = ps_a.tile([128, 256], F32, tag="pj")
                    for k in range(NK):
                        nc.tensor.matmul(
                            ps[:], w1_t[:, k, f * 128:(f + 1) * 128],
                            xf_fm[:, k, jq * 256:(jq + 1) * 256],
                            start=(k == 0), stop=(k == NK - 1))
                    nc.scalar.activation(out=h_s[:, f, :], in_=ps[:], func=AF.Relu,
                                         bias=b1_s[:, l, f:f + 1], scale=1.0)
                for ii in range(2):        # token tiles within the quarter
                    i = jq * 2 + ii
                    ps = ps_a.tile([128, 512], F32, tag="pj")
                    for f in range(NF):
                        nc.tensor.matmul(
                            ps[:], h_s[:, f, ii * 128:(ii + 1) * 128],
                            w2_t[:, f, :],
                            start=(f == 0), stop=False)
                    nc.tensor.matmul(ps[:], ones_s[:],
                                     b2row, start=False, stop=True)
                    nc.vector.tensor_add(x_s[:, i, :], x_s[:, i, :], ps[:])

        # ---------------- decoder ----------------
        xh_fm = fmp.tile([128, NK, T], F32R, tag="fm")
        layernorm_to_fm(xh_fm)

        dw1_t = load_w(wqkvo, d['decw1'][:], NK, D_MODEL, dtype=F32R, tag="w")
        d1_fm = qp.tile([128, NK, T], F32R)
        for m in range(NK):
            for j in range(2):
                ps = ps_a.tile([128, 512], F32, tag="pj")
                for k in range(NK):
                    nc.tensor.matmul(
                        ps[:], dw1_t[:, k, m * 128:(m + 1) * 128],
                        xh_fm[:, k, j * 512:(j + 1) * 512],
                        start=(k == 0), stop=(k == NK - 1))
                prelu_from_psum(d1_fm[:, m, j * 512:(j + 1) * 512], ps[:],
                                alpham1_s[:, 2:3], decb1_s[:, m:m + 1])

        dw2_t = load_w(wqkvo, d['decw2'][:], NK, D_OUT_P, dtype=F32R, tag="w")
        decb2 = bias_row(d['decb2r'][0:1, :])
        for i in range(NT):
            ps = ps_a.tile([128, 512], F32, tag="pj")
            for k in range(NK):
                nc.tensor.matmul(ps[:, 0:D_OUT_P],
                                 d1_fm[:, k, i * 128:(i + 1) * 128],
                                 dw2_t[:, k, :],
                                 start=(k == 0), stop=False)
            nc.tensor.matmul(ps[:, 0:D_OUT_P], ones_s[:],
                             decb2, start=False, stop=True)
            mo_t = io.tile([128, D_MOTION], F32, tag="mo")
            dma(out=mo_t[:], in_=d['motion'][i * 128:(i + 1) * 128, :])
            df = scr.tile([128, D_MOTION], F32, tag="df")
            nc.vector.tensor_sub(df[:], mo_t[:], ps[:, 0:D_MOTION])
            y_t = io.tile([128, D_OUT], F32, tag="y")
            nc.vector.scalar_tensor_tensor(
                out=y_t[:, 0:D_MOTION], in0=df[:], scalar=mask_s[:, i:i + 1],
                in1=ps[:, 0:D_MOTION], op0=ALU.mult, op1=ALU.add)
            nc.scalar.activation(out=y_t[:, D_MOTION:D_OUT],
                                 in_=ps[:, D_MOTION:D_OUT], func=AF.Sigmoid)
            dma(out=d['y'][i * 128:(i + 1) * 128, :], in_=y_t[:])


# --------------------------------------------------------------------------
# public entry point
# --------------------------------------------------------------------------

_NC_CACHE = {}


def _get_nc():
    if 'nc' not in _NC_CACHE:
        nc = build_nc()
        nc.finalize()
        _NC_CACHE['nc'] = nc
    return _NC_CACHE['nc']


def kernel(motion, traj, mask, params, _trace=False):
    from concourse.bass_utils import run_bass_kernel_spmd
    shared, per_core = _host_prep(motion, traj, mask, params)
    in_maps = [{**shared, **pc} for pc in per_core]
    nc = _get_nc()
    res = run_bass_kernel_spmd(nc, in_maps, core_ids=list(range(N_CORES)),
                               trace=_trace)
    if _trace:
        kernel.last_exec_time_ns = res.exec_time_ns
        kernel.last_results = res
    y = np.stack([res.results[c]['y'] for c in range(N_CORES)], axis=0)
    motion_out = np.ascontiguousarray(y[..., :D_MOTION])
    contact = np.ascontiguousarray(y[..., D_MOTION:])
    return motion_out, contact
